# revision 1
# baseline (speedup 1.0000x reference)
"""Trainium2 Bass kernel for CustomDynamicEdgeConv (gnn_message_passing).

Reference computation:
    x_i = x[tgt]; x_j = x[src]
    feat = concat([x_i, x_j - x_i], -1)            # [E, 2D]
    h    = relu(feat @ W1 + b1)                    # [E, H]
    msg  = h @ W2 + b2                             # [E, Do]
    out  = segment_sum(msg, tgt) / (deg + 1e-8)

Algebraic reformulation (node-level matmuls instead of edge-level):
    W1 = [W1a; W1b] (row split at D)
    feat @ W1 = x_i @ W1a + (x_j - x_i) @ W1b = P[tgt] + Q[src]
      with P = x @ (W1a - W1b) + b1   (node-level, [N, H])
           Q = x @ W1b                (node-level, [N, H])
    h_e = relu(P[tgt_e] + Q[src_e])
    sum_e (h_e @ W2 + b2) = (sum_e h_e) @ W2 + deg * b2
    out = (S @ W2) * recip + b2 * (deg * recip),  S = segment_sum(h, tgt)

Sharding: nodes are assigned to 128-node blocks by a degree-balancing
permutation (host side), 16 blocks per core; each core receives exactly the
edges whose (permuted) target lies in its range, so no cross-core reduction
is needed.  Q is computed (replicated) on every core and gathered by src via
multi-queue SWDGE dma_gather (bf16 rows).  P stays resident in SBUF; P[tgt]
is expanded on the PE with an uploaded one-hot (M^T) matmul, and Q_g is
added by an identity-matmul accumulate, so h_pre forms directly in PSUM.
The segment sum runs on the PE as a one-hot matmul per 128-edge tile
accumulating into PSUM per 128-node block.  Host un-permutes the output.
"""
import sys

sys.path.insert(0, "/opt/trn_rl_repo")

import heapq

import numpy as np
import ml_dtypes

import concourse.bacc as bacc
import concourse.mybir as mybir
from concourse.tile import TileContext
from concourse.bass_utils import run_bass_kernel_spmd
from concourse.library_config import mlp

N = 16384        # nodes
D = 256          # input feature dim
H = 512          # hidden dim
DO = 256         # output dim
E = 262144       # edges
NCORES = 8
NPC = N // NCORES          # nodes per core (2048)
NB = NPC // 128            # 128-node blocks per core (16)
CG = 512                   # phase-1 column-group width (4 node tiles)
NCG_Q = N // CG            # 32 column groups for Q
NCG_P = NPC // CG          # 4 column groups for P

f32 = mybir.dt.float32
f32r = mybir.dt.float32r
bf16 = mybir.dt.bfloat16
i16 = mybir.dt.int16

_program_cache: dict = {}


def _build(tpb: int, reps: int = 1, ablate: frozenset = frozenset()):
    """Build the SPMD Bass program. tpb = padded 128-edge tiles per node block.

    reps > 1 wraps the compute in a device-side loop (benchmarking only).
    ablate: subset of {"p1","plaingather","vec","scatter","tail"} (timing).
    """
    nc = bacc.Bacc("TRN2", num_swdge_queues=4)

    xt = nc.dram_tensor("xt", [D, N], bf16, kind="ExternalInput")
    xt_own = nc.dram_tensor("xt_own", [D, NPC], bf16, kind="ExternalInput")
    w1b = nc.dram_tensor("w1b", [D, H], bf16, kind="ExternalInput")
    w1d = nc.dram_tensor("w1d", [D, H], bf16, kind="ExternalInput")
    b1b = nc.dram_tensor("b1b", [128, H], f32, kind="ExternalInput")
    w2 = nc.dram_tensor("w2", [H, DO], f32, kind="ExternalInput")
    b2b = nc.dram_tensor("b2b", [128, DO], f32, kind="ExternalInput")
    iota = nc.dram_tensor("iota", [128, 128], f32, kind="ExternalInput")
    ident = nc.dram_tensor("ident", [128, 128], f32, kind="ExternalInput")
    identb = nc.dram_tensor("identb", [128, 128], bf16, kind="ExternalInput")
    qidx = nc.dram_tensor("qidx", [128, NB * tpb * 8], i16, kind="ExternalInput")
    mtd = nc.dram_tensor("mtd", [128, NB * tpb * 128], bf16, kind="ExternalInput")
    tloc = nc.dram_tensor("tloc", [128, NB * tpb], f32, kind="ExternalInput")
    recip = nc.dram_tensor("recip", [128, NB], f32, kind="ExternalInput")
    gdeg = nc.dram_tensor("gdeg", [128, NB], f32, kind="ExternalInput")

    outd = nc.dram_tensor("outd", [NPC, DO], f32, kind="ExternalOutput")
    qd = nc.dram_tensor("qd", [N, H], bf16, kind="Internal")

    with TileContext(nc) as tc:
        nc.gpsimd.load_library(mlp)
        with tc.tile_pool(name="const", bufs=1) as cpool:
            w1b_sb, w1d_sb = [], []
            for name, dram, dst in (("w1b", w1b, w1b_sb), ("w1d", w1d, w1d_sb)):
                for kk in range(2):
                    t = cpool.tile([128, H], bf16, name=f"{name}_{kk}")
                    nc.sync.dma_start(t[:], dram[kk * 128:(kk + 1) * 128, :])
                    dst.append(t)
            w2_r = []
            for kk in range(4):
                t = cpool.tile([128, DO], f32, name=f"w2_{kk}")
                nc.sync.dma_start(t[:], w2[kk * 128:(kk + 1) * 128, :])
                tr = cpool.tile([128, DO], f32r, name=f"w2r_{kk}")
                nc.vector.tensor_copy(tr[:], t[:])
                w2_r.append(tr)

            b1b_sb = cpool.tile([128, H], f32)
            nc.sync.dma_start(b1b_sb[:], b1b[:])
            b2b_sb = cpool.tile([128, DO], f32)
            nc.sync.dma_start(b2b_sb[:], b2b[:])
            iota_sb = cpool.tile([128, 128], f32)
            nc.sync.dma_start(iota_sb[:], iota[:])
            ident_sb = cpool.tile([128, 128], f32)
            nc.sync.dma_start(ident_sb[:], ident[:])
            identb_sb = cpool.tile([128, 128], bf16)
            nc.sync.dma_start(identb_sb[:], identb[:])
            qidx_sb = cpool.tile([128, NB * tpb * 8], i16)
            nc.sync.dma_start(qidx_sb[:], qidx[:])
            tloc_sb = cpool.tile([128, NB * tpb], f32)
            nc.sync.dma_start(tloc_sb[:], tloc[:])
            recip_sb = cpool.tile([128, NB], f32)
            nc.sync.dma_start(recip_sb[:], recip[:])
            gdeg_sb = cpool.tile([128, NB], f32)
            nc.sync.dma_start(gdeg_sb[:], gdeg[:])
            # P stays SBUF-resident: one [128, H] bf16 tile per node block
            p_own = [cpool.tile([128, H], bf16, name=f"p_own_{b}")
                     for b in range(NB)]

            # ---- Phase 1: Q = x @ W1b (to DRAM), P = x_own @ W1d + b1 (SBUF)
            if reps > 1:
                _loop = tc.For_i(0, reps, 1)
                _loop.__enter__()
            with tc.tile_pool(name="p1", bufs=3) as p1, \
                 tc.tile_pool(name="p1acc", bufs=6, space="PSUM") as p1acc:
                for phase, (ncg, xsrc, wsb) in enumerate((
                        (NCG_Q, xt, w1b_sb),
                        (NCG_P, xt_own, w1d_sb))):
                    if "p1" in ablate:
                        continue
                    for i in range(ncg):
                        xa = p1.tile([128, CG], bf16, tag="xa")
                        nc.sync.dma_start(xa[:], xsrc[0:128, i * CG:(i + 1) * CG])
                        xb = p1.tile([128, CG], bf16, tag="xb")
                        nc.sync.dma_start(xb[:], xsrc[128:256, i * CG:(i + 1) * CG])
                        osb = None
                        if phase == 0:
                            osb = p1.tile([128, CG // 128, H], bf16, tag="osb")
                        for j in range(CG // 128):
                            acc = p1acc.tile([128, H], f32, tag="acc")
                            nc.tensor.matmul(acc[:], xa[:, j * 128:(j + 1) * 128],
                                             wsb[0][:], start=True, stop=False)
                            nc.tensor.matmul(acc[:], xb[:, j * 128:(j + 1) * 128],
                                             wsb[1][:], start=False, stop=True)
                            if phase == 0:
                                nc.any.tensor_copy(osb[:, j, :], acc[:])
                            else:
                                nc.vector.scalar_tensor_tensor(
                                    p_own[i * 4 + j][:], acc[:], 1.0, b1b_sb[:],
                                    mybir.AluOpType.mult, mybir.AluOpType.add)
                        if phase == 0:
                            nc.sync.dma_start(
                                qd[i * CG:(i + 1) * CG, :].rearrange(
                                    "(j p) f -> p j f", p=128),
                                osb[:])

            # ---- Phase 2: Q gather; PE one-hot expand of P + identity-add of
            # Q (h_pre in PSUM); relu; one-hot scatter; final matmul
            with tc.tile_pool(name="qg", bufs=4) as qgp, \
                 tc.tile_pool(name="mt", bufs=2) as mtp, \
                 tc.tile_pool(name="ph", bufs=6) as php, \
                 tc.tile_pool(name="po", bufs=2) as pop, \
                 tc.tile_pool(name="hps", bufs=2, space="PSUM") as hpsp, \
                 tc.tile_pool(name="sps", bufs=2, space="PSUM") as spsp, \
                 tc.tile_pool(name="stp", bufs=2, space="PSUM") as stpp, \
                 tc.tile_pool(name="ops", bufs=2, space="PSUM") as opsp:
                for b in range(NB):
                    qg = qgp.tile([128, tpb, H], bf16, tag="qg")
                    if "plaingather" in ablate:
                        nc.sync.dma_start(
                            qg[:], qd[0:tpb * 128, :].rearrange(
                                "(j p) f -> p j f", p=128))
                    else:
                        nc.gpsimd.dma_gather(
                            qg[:], qd[:], qidx_sb[:, b * tpb * 8:(b + 1) * tpb * 8],
                            tpb * 128, tpb * 128, H, single_packet=False,
                            queue_num=b % 4)
                    mt_sb = mtp.tile([128, tpb * 128], bf16, tag="mt")
                    nc.sync.dma_start(
                        mt_sb[:], mtd[:, b * tpb * 128:(b + 1) * tpb * 128])
                    s_ps = None
                    if "scatter" not in ablate:
                        s_ps = spsp.tile([128, H], f32, tag="s")
                    for kk in range(tpb):
                        h = php.tile([128, H], bf16, tag="h")
                        m = php.tile([128, 128], bf16, tag="m")
                        if "vec" not in ablate:
                            h_ps = hpsp.tile([128, H], f32, tag="hps")
                            nc.tensor.matmul(
                                h_ps[:], mt_sb[:, kk * 128:(kk + 1) * 128],
                                p_own[b][:], start=True, stop=False)
                            nc.tensor.matmul(h_ps[:], identb_sb[:], qg[:, kk, :],
                                             start=False, stop=True)
                            nc.any.tensor_scalar(h[:], h_ps[:], 0.0, None,
                                                 mybir.AluOpType.max)
                            tcol = b * tpb + kk
                            nc.any.tensor_scalar(m[:], iota_sb[:],
                                                 tloc_sb[:, tcol:tcol + 1], None,
                                                 mybir.AluOpType.is_equal)
                        elif "scatter" not in ablate:
                            nc.any.memset(h[:], 0)
                            nc.any.memset(m[:], 0)
                        if "scatter" not in ablate:
                            nc.tensor.matmul(s_ps[:], m[:], h[:],
                                             start=(kk == 0), stop=(kk == tpb - 1))
                    if "tail" in ablate:
                        continue
                    s_sb = pop.tile([128, H], f32, tag="ssb")
                    if "scatter" in ablate:
                        nc.vector.memset(s_sb[:], 0)
                    else:
                        nc.any.tensor_copy(s_sb[:], s_ps[:])
                    o_ps = opsp.tile([128, DO], f32, tag="o")
                    for kk in range(4):
                        st = stpp.tile([128, 128], f32, tag="st")
                        nc.tensor.transpose(st[:], s_sb[:, kk * 128:(kk + 1) * 128],
                                            ident_sb[:])
                        st_r = php.tile([128, 128], f32r, tag="str")
                        nc.vector.tensor_copy(st_r[:], st[:])
                        nc.tensor.matmul(o_ps[:], st_r[:], w2_r[kk][:],
                                         start=(kk == 0), stop=(kk == 3))
                    t1 = pop.tile([128, DO], f32, tag="t1")
                    nc.any.tensor_scalar(t1[:], b2b_sb[:], gdeg_sb[:, b:b + 1],
                                         None, mybir.AluOpType.mult)
                    o_sb = pop.tile([128, DO], f32, tag="osb2")
                    nc.vector.scalar_tensor_tensor(
                        o_sb[:], o_ps[:], recip_sb[:, b:b + 1], t1[:],
                        mybir.AluOpType.mult, mybir.AluOpType.add)
                    nc.sync.dma_start(outd[b * 128:(b + 1) * 128, :], o_sb[:])

            if reps > 1:
                _loop.__exit__(None, None, None)

    nc.compile()
    return nc


def _wrap_idx(flat: np.ndarray) -> np.ndarray:
    """Pack a flat int16 index list into the SWDGE layout [128, len/16]."""
    return np.tile(flat.reshape(-1, 16).T, (8, 1)).copy()


def _balance_nodes(deg: np.ndarray):
    """Assign nodes to 128 blocks of exactly 128 nodes, balancing total degree.

    Returns perm[N]: perm[slot] = original node id; blocks 16c..16c+15
    belong to core c.
    """
    nblocks = N // 128
    order = np.argsort(-deg, kind="stable")
    heap = [(0, 0, blk) for blk in range(nblocks)]   # (edges, nodes, blk)
    heapq.heapify(heap)
    members = [[] for _ in range(nblocks)]
    for node in order:
        w = int(deg[node])
        stash = []
        while True:
            edges, nodes, blk = heapq.heappop(heap)
            if nodes < 128:
                break
            stash.append((edges, nodes, blk))
        members[blk].append(node)
        heapq.heappush(heap, (edges + w, nodes + 1, blk))
        for it in stash:
            heapq.heappush(heap, it)
    perm = np.empty(N, np.int64)
    for blk in range(nblocks):
        assert len(members[blk]) == 128
        perm[blk * 128:(blk + 1) * 128] = members[blk]
    return perm


def _prepare(x, W1, b1, W2, b2, nn_index):
    src = np.asarray(nn_index[0]).astype(np.int64)
    tgt = np.asarray(nn_index[1]).astype(np.int64)
    deg = np.bincount(tgt, minlength=N).astype(np.int64)

    perm = _balance_nodes(deg)              # slot -> node
    inv = np.empty(N, np.int64)             # node -> slot
    inv[perm] = np.arange(N)

    tslot = inv[tgt]                        # permuted targets
    deg_slot = deg[perm].astype(np.float64)
    recip_full = (1.0 / (deg_slot + 1e-8)).astype(np.float32)
    gdeg_full = (deg_slot * recip_full).astype(np.float32)

    blk = tslot >> 7                        # permuted block id (0..127)
    order = np.lexsort((src, blk))
    src_s, tslot_s, blk_s = src[order], tslot[order], blk[order]
    counts = np.bincount(blk_s, minlength=N // 128)
    starts = np.concatenate(([0], np.cumsum(counts)))
    tpb = int(np.ceil(counts.max() / 128))
    pad = tpb * 128

    W1 = np.asarray(W1, np.float32)
    w1b_np = W1[D:].astype(ml_dtypes.bfloat16)
    w1d_np = (W1[:D] - W1[D:]).astype(ml_dtypes.bfloat16)
    b1b_np = np.tile(np.asarray(b1, np.float32)[None, :], (128, 1))
    b2b_np = np.tile(np.asarray(b2, np.float32)[None, :], (128, 1))
    iota_np = np.tile(np.arange(128, dtype=np.float32), (128, 1))
    ident_np = np.eye(128, dtype=np.float32)
    identb_np = np.eye(128, dtype=ml_dtypes.bfloat16)
    xt_np = np.ascontiguousarray(
        np.asarray(x, np.float32).T.astype(ml_dtypes.bfloat16))

    in_maps = []
    for c in range(NCORES):
        qflat = np.zeros((NB, pad), np.int16)
        tl = np.full((NB, pad), -1.0, np.float32)
        for b in range(NB):
            g = c * NB + b               # global (permuted) block
            s, e = starts[g], starts[g + 1]
            n = e - s
            qflat[b, :n] = src_s[s:e].astype(np.int16)
            tl[b, :n] = (tslot_s[s:e] & 127).astype(np.float32)
        # M^T tiles: [128, NB*tpb*128] bf16; column (b*tpb+kk)*128+e one-hot
        mtd_np = np.zeros((128, NB * pad), ml_dtypes.bfloat16)
        cols = np.arange(NB * pad)
        tlf = tl.reshape(-1)
        valid = tlf >= 0
        mtd_np[tlf[valid].astype(np.int64), cols[valid]] = 1
        tloc_np = np.ascontiguousarray(tl.reshape(NB * tpb, 128).T)
        recip_np = np.ascontiguousarray(
            recip_full[c * NPC:(c + 1) * NPC].reshape(NB, 128).T)
        gdeg_np = np.ascontiguousarray(
            gdeg_full[c * NPC:(c + 1) * NPC].reshape(NB, 128).T)
        in_maps.append({
            "xt": xt_np,
            "xt_own": np.ascontiguousarray(xt_np[:, perm[c * NPC:(c + 1) * NPC]]),
            "w1b": w1b_np, "w1d": w1d_np, "b1b": b1b_np,
            "w2": np.asarray(W2, np.float32), "b2b": b2b_np,
            "iota": iota_np, "ident": ident_np, "identb": identb_np,
            "qidx": _wrap_idx(qflat.reshape(-1)),
            "mtd": mtd_np,
            "tloc": tloc_np, "recip": recip_np, "gdeg": gdeg_np,
        })
    return tpb, in_maps, perm


def kernel(x, W1, b1, W2, b2, nn_index, k=None, _trace=False, _tmpdir=None):
    tpb, in_maps, perm = _prepare(x, W1, b1, W2, b2, nn_index)
    if tpb not in _program_cache:
        _program_cache[tpb] = _build(tpb)
    nc = _program_cache[tpb]
    res = run_bass_kernel_spmd(nc, in_maps, core_ids=list(range(NCORES)),
                               trace=_trace, tmpdir=_tmpdir)
    out_perm = np.concatenate([res.results[c]["outd"] for c in range(NCORES)],
                              axis=0)
    out = np.empty_like(out_perm)
    out[perm] = out_perm                    # slot s holds node perm[s]
    if _trace:
        return out.astype(np.float32), res
    return out.astype(np.float32)



# revision 3
# speedup vs baseline: 3.8598x; 3.8598x over previous
"""Trainium2 Bass kernel for CustomDynamicEdgeConv (gnn_message_passing).

Reference computation:
    x_i = x[tgt]; x_j = x[src]
    feat = concat([x_i, x_j - x_i], -1)            # [E, 2D]
    h    = relu(feat @ W1 + b1)                    # [E, H]
    msg  = h @ W2 + b2                             # [E, Do]
    out  = segment_sum(msg, tgt) / (deg + 1e-8)

Algebraic reformulation:
    W1 = [W1a; W1b] (row split at D)
    feat @ W1 = x_i @ (W1a - W1b) + x_j @ W1b = P[tgt] + x_j @ W1b
      with P = x @ (W1a - W1b) + b1   (node-level, [N, H])
    h_e = relu(P[tgt_e] + x[src_e] @ W1b)
    out = (S @ W2) * recip + b2 * (deg * recip),  S = segment_sum(h, tgt)

Design (v6):
  * Nodes are assigned to 128-node blocks by a degree-balancing permutation
    (host side), 16 blocks per core; each core receives exactly the edges
    whose (permuted) target lies in its range — no cross-core reduction.
  * The host pre-gathers x[src] into a dense, transposed, per-tile layout
    (fp8, scaled) so the device does only big sequential DMA reads — no
    SWDGE row-gather.  Q = x_j @ W1b is computed per-edge on the PE with a
    single fp8 DoubleRow matmul (contraction 256), accumulating in PSUM.
  * P stays SBUF-resident per block; P[tgt] is added into the same PSUM
    accumulation with a one-hot (M^T) matmul.
  * relu (PSUM -> SBUF bf16) is split between DVE (cols :384) and ACT
    (cols 384:) so neither engine bottlenecks.
  * The segment sum runs on the PE as a one-hot matmul per 128-edge tile,
    accumulating S per 128-node block in PSUM.
  * fp8 scale trick: W1b and P are scaled by 32 (fp8e4 subnormal dodge);
    S comes out scaled by 32; 1/32 is folded into recip for the W2 stage.
  * Tail: S (bf16) is written to DRAM, read back through the hardware
    DMA-transpose, and projected with W2 in dense bf16 matmuls.
  * Host un-permutes the output.
"""
import sys

sys.path.insert(0, "/opt/trn_rl_repo")

import heapq

import numpy as np
import ml_dtypes

import concourse.bacc as bacc
import concourse.mybir as mybir
from concourse.tile import TileContext
from concourse.bass_utils import run_bass_kernel_spmd

N = 16384        # nodes
D = 256          # input feature dim
H = 512          # hidden dim
DO = 256         # output dim
E = 262144       # edges
NCORES = 8
NPC = N // NCORES          # nodes per core (2048)
NB = NPC // 128            # 128-node blocks per core (16)
CG = 512                   # phase-1 column-group width (4 node tiles)
NCG_P = NPC // CG          # 4 column groups for P
RS = 384                   # relu split point: DVE does [:RS], ACT does [RS:]

USE_DR = True              # fp8 DoubleRow for the per-edge Q matmul
FSCALE = 32.0              # fp8 scale for W1b / P (power of two)

f32 = mybir.dt.float32
bf16 = mybir.dt.bfloat16
f8e4 = mybir.dt.float8e4

_program_cache: dict = {}


def _build(tpb: int, reps: int = 1, ablate: frozenset = frozenset(),
           use_dr: bool = USE_DR):
    """Build the SPMD Bass program. tpb = padded 128-edge tiles per node block.

    reps > 1 wraps the compute in a device-side loop (benchmarking only).
    ablate: subset of {"p1","h","relu","scatter","tail"} (timing only).
    """
    nc = bacc.Bacc("TRN2")
    G = NB * tpb                       # edge tiles per core

    xt_own = nc.dram_tensor("xt_own", [D, NPC], bf16, kind="ExternalInput")
    w1d = nc.dram_tensor("w1d", [D, H], bf16, kind="ExternalInput")
    if use_dr:
        w1bd = nc.dram_tensor("w1bd", [128, 2, H], f8e4, kind="ExternalInput")
        xgt = nc.dram_tensor("xgt", [128, G, 2, 128], f8e4,
                             kind="ExternalInput")
    else:
        w1bd = nc.dram_tensor("w1bd", [D, H], bf16, kind="ExternalInput")
        xgt = nc.dram_tensor("xgt", [128, G, 2, 128], bf16,
                             kind="ExternalInput")
    b1b = nc.dram_tensor("b1b", [128, H], f32, kind="ExternalInput")
    w2b = nc.dram_tensor("w2b", [128, 4, DO], bf16, kind="ExternalInput")
    b2b = nc.dram_tensor("b2b", [128, DO], f32, kind="ExternalInput")
    iota = nc.dram_tensor("iota", [128, 128], f32, kind="ExternalInput")
    mtd = nc.dram_tensor("mtd", [128, G * 128], bf16, kind="ExternalInput")
    tloc = nc.dram_tensor("tloc", [128, G], f32, kind="ExternalInput")
    recs = nc.dram_tensor("recs", [128, NB], f32, kind="ExternalInput")
    gdeg = nc.dram_tensor("gdeg", [128, NB], f32, kind="ExternalInput")

    outd = nc.dram_tensor("outd", [NPC, DO], f32, kind="ExternalOutput")
    sd = nc.dram_tensor("sd", [NPC, H], bf16, kind="Internal")

    relu_t = mybir.ActivationFunctionType.Relu
    mult = mybir.AluOpType.mult
    add = mybir.AluOpType.add
    amax = mybir.AluOpType.max
    iseq = mybir.AluOpType.is_equal

    with TileContext(nc) as tc:
        with tc.tile_pool(name="const", bufs=1) as cpool:
            w1d_sb = []
            for kk in range(2):
                t = cpool.tile([128, H], bf16, name=f"w1d_{kk}")
                nc.sync.dma_start(t[:], w1d[kk * 128:(kk + 1) * 128, :])
                w1d_sb.append(t)
            if use_dr:
                w1b_sb = cpool.tile([128, 2, H], f8e4, name="w1b")
                nc.sync.dma_start(w1b_sb[:], w1bd[:])
            else:
                w1b_sb = []
                for kk in range(2):
                    t = cpool.tile([128, H], bf16, name=f"w1b_{kk}")
                    nc.sync.dma_start(t[:], w1bd[kk * 128:(kk + 1) * 128, :])
                    w1b_sb.append(t)
            w2_sb = cpool.tile([128, 4, DO], bf16, name="w2")
            nc.sync.dma_start(w2_sb[:], w2b[:])
            b1b_sb = cpool.tile([128, H], f32)
            nc.sync.dma_start(b1b_sb[:], b1b[:])
            b2b_sb = cpool.tile([128, DO], f32)
            nc.sync.dma_start(b2b_sb[:], b2b[:])
            iota_sb = cpool.tile([128, 128], f32)
            nc.sync.dma_start(iota_sb[:], iota[:])
            tloc_sb = cpool.tile([128, G], f32)
            nc.sync.dma_start(tloc_sb[:], tloc[:])
            recs_sb = cpool.tile([128, NB], f32)
            nc.sync.dma_start(recs_sb[:], recs[:])
            gdeg_sb = cpool.tile([128, NB], f32)
            nc.sync.dma_start(gdeg_sb[:], gdeg[:])
            # P stays SBUF-resident: one [128, H] bf16 tile per node block
            p_own = [cpool.tile([128, H], bf16, name=f"p_own_{b}")
                     for b in range(NB)]

            if reps > 1:
                _loop = tc.For_i(0, reps, 1)
                _loop.__enter__()

            # ---- Phase 1: P = FSCALE * (x_own @ (W1a-W1b) + b1)  (SBUF)
            if "p1" not in ablate:
                with tc.tile_pool(name="p1", bufs=2) as p1, \
                     tc.tile_pool(name="p1acc", bufs=4, space="PSUM") as p1acc:
                    for i in range(NCG_P):
                        xa = p1.tile([128, CG], bf16, tag="xa")
                        nc.sync.dma_start(xa[:], xt_own[0:128,
                                                        i * CG:(i + 1) * CG])
                        xb = p1.tile([128, CG], bf16, tag="xb")
                        nc.sync.dma_start(xb[:], xt_own[128:256,
                                                        i * CG:(i + 1) * CG])
                        for j in range(CG // 128):
                            acc = p1acc.tile([128, H], f32, tag="acc")
                            nc.tensor.matmul(acc[:],
                                             xa[:, j * 128:(j + 1) * 128],
                                             w1d_sb[0][:],
                                             start=True, stop=False)
                            nc.tensor.matmul(acc[:],
                                             xb[:, j * 128:(j + 1) * 128],
                                             w1d_sb[1][:],
                                             start=False, stop=True)
                            nc.vector.scalar_tensor_tensor(
                                p_own[i * 4 + j][:], acc[:], FSCALE,
                                b1b_sb[:], mult, add)

            # ---- Phase 2: per-edge Q matmul + P expand -> relu -> scatter
            with tc.tile_pool(name="xgp", bufs=2) as xgp, \
                 tc.tile_pool(name="mtp", bufs=2) as mtp, \
                 tc.tile_pool(name="php", bufs=6) as php, \
                 tc.tile_pool(name="ssb", bufs=2) as ssbp, \
                 tc.tile_pool(name="hps", bufs=3, space="PSUM") as hpsp, \
                 tc.tile_pool(name="sps", bufs=2, space="PSUM") as spsp:
                for b in range(NB):
                    xg = xgp.tile([128, tpb, 2, 128],
                                  f8e4 if use_dr else bf16, tag="xg")
                    nc.sync.dma_start(xg[:], xgt[:, b * tpb:(b + 1) * tpb])
                    mt_sb = mtp.tile([128, tpb * 128], bf16, tag="mt")
                    nc.sync.dma_start(
                        mt_sb[:], mtd[:, b * tpb * 128:(b + 1) * tpb * 128])
                    s_ps = spsp.tile([128, H], f32, tag="s")
                    for kk in range(tpb):
                        h = php.tile([128, H], bf16, tag="h")
                        m = php.tile([128, 128], bf16, tag="m")
                        h_ps = hpsp.tile([128, H], f32, tag="hps")
                        if "h" not in ablate:
                            if use_dr:
                                nc.tensor.matmul(
                                    h_ps[:], xg[:, kk], w1b_sb[:],
                                    perf_mode=mybir.MatmulPerfMode.DoubleRow,
                                    start=True, stop=False)
                            else:
                                nc.tensor.matmul(h_ps[:], xg[:, kk, 0],
                                                 w1b_sb[0][:],
                                                 start=True, stop=False)
                                nc.tensor.matmul(h_ps[:], xg[:, kk, 1],
                                                 w1b_sb[1][:],
                                                 start=False, stop=False)
                            nc.tensor.matmul(
                                h_ps[:],
                                mt_sb[:, kk * 128:(kk + 1) * 128],
                                p_own[b][:],
                                start=False, stop=True)
                        if "relu" in ablate:
                            nc.vector.memset(h[:], 0)
                        else:
                            nc.vector.tensor_scalar(h[:, 0:RS], h_ps[:, 0:RS],
                                                    0.0, None, amax)
                            nc.scalar.activation(h[:, RS:H], h_ps[:, RS:H],
                                                 relu_t)
                        tcol = b * tpb + kk
                        nc.any.tensor_scalar(m[:], iota_sb[:],
                                             tloc_sb[:, tcol:tcol + 1], None,
                                             iseq)
                        if "scatter" not in ablate:
                            nc.tensor.matmul(s_ps[:], m[:], h[:],
                                             start=(kk == 0),
                                             stop=(kk == tpb - 1))
                    s_sb = ssbp.tile([128, H], bf16, tag="ssb")
                    if "scatter" in ablate:
                        nc.vector.memset(s_sb[:], 0)
                    else:
                        nc.vector.tensor_copy(s_sb[:, 0:RS], s_ps[:, 0:RS])
                        nc.scalar.copy(s_sb[:, RS:H], s_ps[:, RS:H])
                    nc.sync.dma_start(sd[b * 128:(b + 1) * 128, :], s_sb[:])

            # ---- Phase 3: out = (S @ W2) * recip/FSCALE + b2 * (deg*recip)
            if "tail" not in ablate:
                with tc.tile_pool(name="p3", bufs=1) as p3p, \
                     tc.tile_pool(name="pop", bufs=4) as pop, \
                     tc.tile_pool(name="ops", bufs=4, space="PSUM") as opsp:
                    sdT = []
                    for c in range(4):
                        t = p3p.tile([128, NPC], bf16, name=f"sdT_{c}")
                        nc.sync.dma_start(
                            t[:], sd[:, c * 128:(c + 1) * 128],
                            transpose=True)
                        sdT.append(t)
                    for tt in range(NB):
                        o_ps = opsp.tile([128, DO], f32, tag="o")
                        for c in range(4):
                            nc.tensor.matmul(
                                o_ps[:],
                                sdT[c][:, tt * 128:(tt + 1) * 128],
                                w2_sb[:, c], start=(c == 0), stop=(c == 3))
                        t1 = pop.tile([128, DO], f32, tag="t1")
                        nc.any.tensor_scalar(t1[:], b2b_sb[:],
                                             gdeg_sb[:, tt:tt + 1], None,
                                             mult)
                        o_sb = pop.tile([128, DO], f32, tag="osb")
                        nc.vector.scalar_tensor_tensor(
                            o_sb[:], o_ps[:], recs_sb[:, tt:tt + 1], t1[:],
                            mult, add)
                        nc.sync.dma_start(outd[tt * 128:(tt + 1) * 128, :],
                                          o_sb[:])

            if reps > 1:
                _loop.__exit__(None, None, None)

    nc.compile()
    return nc


def _balance_nodes(deg: np.ndarray):
    """Assign nodes to 128 blocks of exactly 128 nodes, balancing total degree.

    Returns perm[N]: perm[slot] = original node id; blocks 16c..16c+15
    belong to core c.
    """
    nblocks = N // 128
    order = np.argsort(-deg, kind="stable")
    heap = [(0, 0, blk) for blk in range(nblocks)]   # (edges, nodes, blk)
    heapq.heapify(heap)
    members = [[] for _ in range(nblocks)]
    for node in order:
        w = int(deg[node])
        stash = []
        while True:
            edges, nodes, blk = heapq.heappop(heap)
            if nodes < 128:
                break
            stash.append((edges, nodes, blk))
        members[blk].append(node)
        heapq.heappush(heap, (edges + w, nodes + 1, blk))
        for it in stash:
            heapq.heappush(heap, it)
    perm = np.empty(N, np.int64)
    for blk in range(nblocks):
        assert len(members[blk]) == 128
        perm[blk * 128:(blk + 1) * 128] = members[blk]
    return perm


def _prepare(x, W1, b1, W2, b2, nn_index, use_dr: bool = USE_DR):
    src = np.asarray(nn_index[0]).astype(np.int64)
    tgt = np.asarray(nn_index[1]).astype(np.int64)
    deg = np.bincount(tgt, minlength=N).astype(np.int64)

    perm = _balance_nodes(deg)              # slot -> node
    inv = np.empty(N, np.int64)             # node -> slot
    inv[perm] = np.arange(N)

    tslot = inv[tgt]                        # permuted targets
    deg_slot = deg[perm].astype(np.float64)
    recip_full = (1.0 / (deg_slot + 1e-8)).astype(np.float32)
    gdeg_full = (deg_slot * recip_full).astype(np.float32)

    blk = tslot >> 7                        # permuted block id (0..127)
    order = np.lexsort((src, blk))
    src_s, tslot_s, blk_s = src[order], tslot[order], blk[order]
    counts = np.bincount(blk_s, minlength=N // 128)
    starts = np.concatenate(([0], np.cumsum(counts)))
    tpb = int(np.ceil(counts.max() / 128))
    pad = tpb * 128
    G = NB * tpb

    fscale = FSCALE if use_dr else 1.0
    W1 = np.asarray(W1, np.float32)
    f8 = mybir.dt.np(f8e4)
    w1d_np = (W1[:D] - W1[D:]).astype(ml_dtypes.bfloat16)
    W1b = W1[D:]
    if use_dr:
        # [128, 2, H]: [p, t, :] = fscale * W1b[t*128+p, :]
        w1bd_np = np.ascontiguousarray(
            (W1b * fscale).reshape(2, 128, H).transpose(1, 0, 2)).astype(f8)
    else:
        w1bd_np = W1b.astype(ml_dtypes.bfloat16)
    b1b_np = np.tile(np.asarray(b1, np.float32)[None, :] * fscale, (128, 1))
    b2b_np = np.tile(np.asarray(b2, np.float32)[None, :], (128, 1))
    # [128, 4, DO]: [p, c, :] = W2[c*128+p, :]
    w2b_np = np.ascontiguousarray(
        np.asarray(W2, np.float32).reshape(4, 128, DO).transpose(1, 0, 2)
    ).astype(ml_dtypes.bfloat16)
    iota_np = np.tile(np.arange(128, dtype=np.float32), (128, 1))

    x_np = np.asarray(x, np.float32)
    xt_np = np.ascontiguousarray(x_np.T.astype(ml_dtypes.bfloat16))
    # transposed x for the pre-gather (quantized once, [D, N])
    xT_q = np.ascontiguousarray(x_np.T.astype(f8 if use_dr else
                                              ml_dtypes.bfloat16))

    in_maps = []
    for c in range(NCORES):
        sflat = np.zeros((NB, pad), np.int64)
        tl = np.full((NB, pad), -1.0, np.float32)
        for b in range(NB):
            g = c * NB + b               # global (permuted) block
            s, e = starts[g], starts[g + 1]
            n = e - s
            sflat[b, :n] = src_s[s:e]
            tl[b, :n] = (tslot_s[s:e] & 127).astype(np.float32)
        # pre-gathered transposed x[src]: [128, G, 2, 128]
        # [p, g, t, j] = x[src[g*128+j], t*128+p]
        A = xT_q[:, sflat.reshape(-1)]                  # [256, G*128]
        xgt_np = np.ascontiguousarray(
            A.reshape(2, 128, G, 128).transpose(1, 2, 0, 3))
        # M^T tiles: [128, G*128]; column g*128+e one-hot at row tloc
        mtd_np = np.zeros((128, G * 128), ml_dtypes.bfloat16)
        cols = np.arange(G * 128)
        tlf = tl.reshape(-1)
        valid = tlf >= 0
        mtd_np[tlf[valid].astype(np.int64), cols[valid]] = 1
        tloc_np = np.ascontiguousarray(tl.reshape(G, 128).T)
        recs_np = np.ascontiguousarray(
            (recip_full[c * NPC:(c + 1) * NPC] / fscale).reshape(NB, 128).T)
        gdeg_np = np.ascontiguousarray(
            gdeg_full[c * NPC:(c + 1) * NPC].reshape(NB, 128).T)
        in_maps.append({
            "xt_own": np.ascontiguousarray(
                xt_np[:, perm[c * NPC:(c + 1) * NPC]]),
            "w1d": w1d_np, "w1bd": w1bd_np, "b1b": b1b_np,
            "w2b": w2b_np, "b2b": b2b_np, "iota": iota_np,
            "xgt": xgt_np, "mtd": mtd_np, "tloc": tloc_np,
            "recs": recs_np, "gdeg": gdeg_np,
        })
    return tpb, in_maps, perm


def kernel(x, W1, b1, W2, b2, nn_index, k=None, _trace=False, _tmpdir=None):
    tpb, in_maps, perm = _prepare(x, W1, b1, W2, b2, nn_index)
    key = (tpb, USE_DR)
    if key not in _program_cache:
        _program_cache[key] = _build(tpb, use_dr=USE_DR)
    nc = _program_cache[key]
    res = run_bass_kernel_spmd(nc, in_maps, core_ids=list(range(NCORES)),
                               trace=_trace, tmpdir=_tmpdir)
    out_perm = np.concatenate([res.results[c]["outd"] for c in range(NCORES)],
                              axis=0)
    out = np.empty_like(out_perm)
    out[perm] = out_perm                    # slot s holds node perm[s]
    if _trace:
        return out.astype(np.float32), res
    return out.astype(np.float32)
